# revision 10
# baseline (speedup 1.0000x reference)
"""Bilinear grid sample on 8 Trainium2 NeuronCores.

Data-parallel over batch: each core handles 2 of the 16 batches.
Per batch: points are processed 128 at a time (one per SBUF partition).
A pair of indirect DMAs gathers, per point, the two needed image rows
(h_floor and h_floor+1) — both from the SAME per-point top-row index,
the bottom via element_offset=W*C — each as 2 adjacent w-columns x 256
channels (512 contiguous floats = 2KiB per descriptor).

Interpolation is the factored 3-op form
  out = (1-mx)(1-my) * [ (p1 + r*p2) + q*(p3 + r*p4) ],
  r = mx/(1-mx), q = my/(1-my)
so the DVE chain per 128-point tile is one 512-elem and two 256-elem
ops (plus a 1-elem sem-observing touch).  This keeps DVE well below
the Pool engine's ~1.2us/DMA SWDGE descriptor-generation cost, which
is the hard floor for indirect gathers on this hardware (1 index per
partition per DMA; the batched ANT gather ucode is not available in
this runtime image).
"""

import numpy as np

import concourse.bass as bass
import concourse.mybir as mybir
import concourse.tile as tile
from concourse.bass_utils import run_bass_kernel_spmd

B, H, W, C, P = 16, 128, 128, 256, 8192
NCORES = 8
BPC = B // NCORES        # batches per core
PTILE = 128              # points per gather tile (one per partition)
TPB = P // PTILE         # gather tiles per batch
OCHUNK = 16              # gather tiles per output store

_f32 = mybir.dt.float32
_i32 = mybir.dt.int32
_mul = mybir.AluOpType.mult
_add = mybir.AluOpType.add
_sub = mybir.AluOpType.subtract


def build_nc() -> bass.Bass:
    nc = bass.Bass("TRN2", num_swdge_queues=2,
                   dynamic_dma_scratch_size=32768)
    x = nc.dram_tensor("x", [BPC * H * W, C], _f32, kind="ExternalInput")
    idx = nc.dram_tensor("idx", [BPC * P, 2], _f32, kind="ExternalInput")
    out = nc.dram_tensor("out", [BPC * P, C], _f32, kind="ExternalOutput")

    kpp = P // PTILE     # points per partition per batch (64)

    with tile.TileContext(nc) as tc:
        with (
            tc.tile_pool(name="ip", bufs=2) as ip,
            tc.tile_pool(name="gp", bufs=6) as gp,
            tc.tile_pool(name="wp", bufs=4) as wp,
            tc.tile_pool(name="op", bufs=2) as op,
        ):
            for lb in range(BPC):
                # --- index prep: [128, 2*kpp] raw (h,w) pairs, point
                # (partition p, slot t) = global point p*kpp + t
                raw = ip.tile([PTILE, 2 * kpp], _f32, tag="raw")
                nc.sync.dma_start(
                    raw[:],
                    idx[lb * P:(lb + 1) * P, :].rearrange(
                        "(p k) c -> p (k c)", p=PTILE
                    ),
                )
                # floor via the round-to-nearest magic constant: rnd = RN(x),
                # flr = rnd - (rnd > x); exact in f32 for x in [0, 2^23).
                rnd = ip.tile([PTILE, 2 * kpp], _f32, tag="rnd")
                nc.vector.tensor_scalar(
                    rnd[:], raw[:], 8388608.0, 8388608.0, _add, _sub
                )
                gtm = ip.tile([PTILE, 2 * kpp], _f32, tag="gtm")
                nc.vector.tensor_tensor(gtm[:], rnd[:], raw[:],
                                        mybir.AluOpType.is_gt)
                flr = ip.tile([PTILE, 2 * kpp], _f32, tag="flr")
                nc.vector.tensor_tensor(flr[:], rnd[:], gtm[:], _sub)
                mu = ip.tile([PTILE, 2 * kpp], _f32, tag="mu")
                nc.vector.tensor_tensor(mu[:], raw[:], flr[:], _sub)
                # top row id (fp32, exact): hf*W + wf  (+ lb*H*W batch base)
                topf = ip.tile([PTILE, kpp], _f32, tag="topf")
                nc.vector.scalar_tensor_tensor(
                    topf[:], flr[:, 0::2], float(W), flr[:, 1::2], _mul, _add
                )
                ids = ip.tile([PTILE, kpp], _i32, tag="ids")
                nc.vector.tensor_scalar(
                    ids[:], topf[:], float(lb * H * W), None, _add
                )
                # weights: mx = frac along h, my = frac along w
                # r = mx/(1-mx), q = my/(1-my), sc = (1-mx)(1-my)
                mx = mu[:, 0::2]
                my = mu[:, 1::2]
                omx = ip.tile([PTILE, kpp], _f32, tag="omx")
                nc.vector.tensor_scalar(omx[:], mx, -1.0, 1.0, _mul, _add)
                omy = ip.tile([PTILE, kpp], _f32, tag="omy")
                nc.vector.tensor_scalar(omy[:], my, -1.0, 1.0, _mul, _add)
                rmx = ip.tile([PTILE, kpp], _f32, tag="rmx")
                nc.vector.reciprocal(rmx[:], omx[:])
                rr = ip.tile([PTILE, kpp], _f32, tag="rr")
                nc.vector.tensor_tensor(rr[:], mx, rmx[:], _mul)
                rmy = ip.tile([PTILE, kpp], _f32, tag="rmy")
                nc.vector.reciprocal(rmy[:], omy[:])
                qq = ip.tile([PTILE, kpp], _f32, tag="qq")
                nc.vector.tensor_tensor(qq[:], my, rmy[:], _mul)
                sc = ip.tile([PTILE, kpp], _f32, tag="sc")
                nc.vector.tensor_tensor(sc[:], omx[:], omy[:], _mul)

                # --- per 128-point tile: gather + interpolate
                for t in range(TPB):
                    # HW indirect DMA semantics: one index per partition,
                    # filling that partition's whole dest row contiguously.
                    gt_ = gp.tile([PTILE, 2 * C], _f32, tag="gt")
                    nc.gpsimd.indirect_dma_start(
                        out=gt_[:],
                        out_offset=None,
                        in_=x[:],
                        in_offset=bass.IndirectOffsetOnAxis(
                            ap=ids[:, t:t + 1], axis=0
                        ),
                    )
                    gb = gp.tile([PTILE, 2 * C], _f32, tag="gb")
                    nc.gpsimd.indirect_dma_start(
                        out=gb[:],
                        out_offset=None,
                        in_=x[:],
                        in_offset=bass.IndirectOffsetOnAxis(
                            ap=ids[:, t:t + 1], axis=0
                        ),
                        element_offset=W * C,
                    )
                    # Touch gt_ on DVE so its completion sem is observed
                    # before uf, which then only needs to wait on gb
                    # (single wait slot per instruction).
                    tch = wp.tile([PTILE, 1], _f32, tag="tch")
                    nc.vector.tensor_copy(tch[:], gt_[:, :1])
                    # uf = top + r*bottom = [p1 + r*p2 | p3 + r*p4]
                    uf = wp.tile([PTILE, 2 * C], _f32, tag="uf")
                    nc.vector.scalar_tensor_tensor(
                        uf[:], gb[:], rr[:, t:t + 1], gt_[:], _mul, _add
                    )
                    # t2 = left + q*right
                    t2 = wp.tile([PTILE, C], _f32, tag="t2")
                    nc.vector.scalar_tensor_tensor(
                        t2[:], uf[:, C:], qq[:, t:t + 1], uf[:, :C], _mul, _add
                    )
                    # out = sc * t2
                    if t % OCHUNK == 0:
                        och = op.tile([PTILE, OCHUNK * C], _f32, tag="och")
                    j = t % OCHUNK
                    nc.vector.tensor_scalar(
                        och[:, j * C:(j + 1) * C], t2[:], sc[:, t:t + 1],
                        None, _mul
                    )
                    if j == OCHUNK - 1:
                        c0 = (t // OCHUNK) * OCHUNK
                        dst = out[lb * P:(lb + 1) * P, :].rearrange(
                            "(p k) c -> p (k c)", p=PTILE
                        )[:, c0 * C:(c0 + OCHUNK) * C]
                        nc.sync.dma_start(dst, och[:])
                        # Touch the chunk on DVE after the store so the DVE
                        # proc observes the store's completion sem: the next
                        # writer of this slot then needs no extra wait, and
                        # the tail drain's DMA waits become redundant.
                        nc.vector.memset(och[:, :1], 0.0)

    _legalize_waits(nc)
    _alternate_gather_queues(nc)
    return nc


def _alternate_gather_queues(nc: bass.Bass) -> None:
    """Send every other indirect gather to the second SWDGE queue so the
    two GpSimd SWDGE contexts can overlap descriptor generation."""
    g = 0
    for bb in nc.m.functions[0].blocks:
        for ins in bb.instructions:
            if (type(ins).__name__ == "InstDMACopy"
                    and getattr(ins, "queue", None) == "qPoolDynamic"):
                if g % 2 == 1:
                    ins.queue = "qPoolDynamic1"
                g += 1


def _legalize_waits(nc: bass.Bass) -> None:
    """Every instruction has a single sync-wait slot in this walrus codegen.
    Tile emits per-proc minimal waits but is not transitively minimal:
    DMA-completion waits show up alongside an engine wait that already
    implies them (slot-reuse WAR on the readers implies the WAW on the old
    DMA; the tail drain's DVE wait implies every DMA sem because each DMA
    sem is observed by DVE/ACT and ACT by the tail DVE touch).  Keep a
    single engine wait in those cases, preferring DVE."""
    for bb in nc.m.functions[0].blocks:
        for ins in bb.instructions:
            si = ins.sync_info
            if si is None or len(si.on_wait) <= 1:
                continue
            kind = type(ins).__name__
            assert kind in ("InstDMACopy", "InstDrain"), (ins.name, kind)
            keep = [w for w in si.on_wait if not w.ant_name.startswith("DMA")]
            drop = [w for w in si.on_wait if w.ant_name.startswith("DMA")]
            assert all(w.ant_name.startswith("DMASW") or
                       w.ant_name.startswith("DMAHW") for w in drop), si.on_wait
            assert len(keep) >= 1, (ins.name, si.on_wait)
            if len(keep) > 1:
                assert kind == "InstDrain", (ins.name, si.on_wait)
                dve = [w for w in keep if w.ant_name.startswith("DVE")]
                assert len(dve) == 1, (ins.name, si.on_wait)
                keep = dve
            si.on_wait = keep


_NC = None


def _get_nc() -> bass.Bass:
    global _NC
    if _NC is None:
        _NC = build_nc()
    return _NC


def kernel(in_tensor: np.ndarray, indices: np.ndarray) -> np.ndarray:
    nc = _get_nc()
    in_maps = []
    for i in range(NCORES):
        in_maps.append(
            {
                "x": np.ascontiguousarray(
                    in_tensor[i * BPC:(i + 1) * BPC], dtype=np.float32
                ).reshape(BPC * H * W, C),
                "idx": np.ascontiguousarray(
                    indices[i * BPC:(i + 1) * BPC], dtype=np.float32
                ).reshape(BPC * P, 2),
            }
        )
    res = run_bass_kernel_spmd(nc, in_maps, core_ids=list(range(NCORES)))
    return np.concatenate(
        [res.results[i]["out"].reshape(BPC, P, C) for i in range(NCORES)], axis=0
    )
